# revision 9
# baseline (speedup 1.0000x reference)
"""ArcFace loss kernel for Trainium2, class-sharded across 8 NeuronCores.

Strategy (vocab/tensor parallel per the module's own sharding):
  - Shard the class axis of `weights` (100000 classes -> 8 x 12800, zero-padded).
  - Each core: normalize its weight shard on device (squares -> ones-matmul
    partition-reduce -> rsqrt via exp(-0.5*ln)), then bf16 matmul
    cos[b, c] = xn . wn with x replicated, then ScalarE exp(64*cos - 64)
    with accum_out producing per-row partial sums of exp.
  - Host: sum the 8 partial exp-sums (f64), fix up the 512 target-class
    entries with the ArcFace margin (cos(theta+m) correction), and take the
    mean cross-entropy.  A fixed shift of -64 (= -S, since cos <= 1) replaces
    the usual running max, so no cross-core max reduction is needed; all
    exp values stay inside normal fp32 range.
"""

import math

import ml_dtypes
import numpy as np

# Problem constants (hardcoded per contract; kernel.py must be self-contained).
B = 512  # batch
D = 512  # feature dim
C = 100000  # classes
S = 64.0
MARGIN = 0.5
COS_M = math.cos(MARGIN)
SIN_M = math.sin(MARGIN)
TH = math.cos(math.pi - MARGIN)
MM = math.sin(math.pi - MARGIN) * MARGIN

NCORES = 8
CH = 512  # classes per chunk (one PSUM bank of fp32)
NCH = 25  # chunks per core
CSH = CH * NCH  # 12800 padded classes per core
CPAD = CSH * NCORES  # 102400
KB = D // 128  # 4 contraction blocks
NB = B // 128  # 4 batch blocks
SHIFT = 64.0  # fixed logsumexp shift (logits = S*cos <= 64)

_CACHE = {}


def _build_nc(repeat=1):
    import concourse.tile as tile
    from concourse import bacc, mybir

    nc = bacc.Bacc(
        "TRN2",
        target_bir_lowering=False,
        debug=False,
        enable_asserts=False,
        num_devices=NCORES,
    )
    bf16 = mybir.dt.bfloat16
    f32 = mybir.dt.float32

    # xnt[d, b] = normalized-x transposed; wt[p, j, k, c] = wn-shard laid out so
    # each 512-class chunk is one contiguous 4KB run per partition.
    xnt = nc.dram_tensor("xnt", [D, B], bf16, kind="ExternalInput").ap()
    wt = nc.dram_tensor("wt", [128, NCH, KB, CH], bf16, kind="ExternalInput").ap()
    s_out = nc.dram_tensor("s_out", [NB, 128], f32, kind="ExternalOutput").ap()

    from contextlib import ExitStack, nullcontext

    with tile.TileContext(nc) as tc, ExitStack() as ctx:
        singles = ctx.enter_context(tc.tile_pool(name="singles", bufs=1))
        wpool = ctx.enter_context(tc.tile_pool(name="wpool", bufs=3))
        qpool = ctx.enter_context(tc.tile_pool(name="qpool", bufs=2))
        rwpool = ctx.enter_context(tc.tile_pool(name="rwpool", bufs=2))
        tpool = ctx.enter_context(tc.tile_pool(name="tpool", bufs=2))
        escrp = ctx.enter_context(tc.tile_pool(name="escr", bufs=4))
        wnpool = ctx.enter_context(tc.tile_pool(name="wnpool", bufs=1))
        psn = ctx.enter_context(tc.tile_pool(name="psn", bufs=2, space="PSUM"))
        psm = ctx.enter_context(tc.tile_pool(name="psm", bufs=4, space="PSUM"))

        ctx.enter_context(tc.For_i(0, repeat, 1) if repeat > 1 else nullcontext())

        # x (stationary operand of every matmul): [p, k, b], b-contiguous.
        xs = singles.tile([128, KB, B], bf16)
        nc.sync.dma_start(out=xs[:], in_=xnt.rearrange("(k p) b -> p k b", p=128))

        ones_t = singles.tile([128, 128], bf16)
        nc.vector.memset(ones_t[:], 1.0)

        # per-partition bias vectors (activation bias must be an AP)
        eps_b = singles.tile([128, 1], f32)
        nc.vector.memset(eps_b[:], 1e-12)
        zero_b = singles.tile([128, 1], f32)
        nc.vector.memset(zero_b[:], 0.0)
        nshift_b = singles.tile([128, 1], f32)
        nc.vector.memset(nshift_b[:], -SHIFT)

        # accum_out landing area: one f32 scalar per (batch row, chunk).
        s_parts = singles.tile([128, NB * NCH], f32)

        # Per-chunk resident normalized weights (distinct tiles -> no false WAR).
        wns = [
            wnpool.tile([128, KB, CH], bf16, tag=f"wn{j}", name=f"wn{j}")
            for j in range(NCH)
        ]

        for j in range(NCH):
            wtile = wpool.tile([128, KB, CH], bf16, tag="wt")
            nc.sync.dma_start(out=wtile[:], in_=wt[:, j, :, :])

            # q = w*w (bf16, fp32-accurate enough: norm err ~0.4%/sqrt(512))
            q = qpool.tile([128, KB, CH], bf16, tag="q")
            nc.vector.tensor_mul(q[:], wtile[:], wtile[:])

            # n2[c] broadcast to all 128 partitions via ones.T @ q
            pn = psn.tile([128, CH], f32, tag="pn")
            for k in range(KB):
                nc.tensor.matmul(
                    pn[:],
                    lhsT=ones_t[:],
                    rhs=q[:, k, :],
                    start=(k == 0),
                    stop=(k == KB - 1),
                )

            # rw = (n2 + eps)^-0.5 = exp(-0.5 * ln(n2 + eps)); Ln+Exp share one
            # ACT table set (natural_log_exp) so no table thrash with main exp.
            lnt = tpool.tile([128, CH], f32, tag="lnt")
            nc.scalar.activation(
                lnt[:], pn[:], mybir.ActivationFunctionType.Ln, bias=eps_b[:], scale=1.0
            )
            rw = rwpool.tile([128, CH], bf16, tag="rw")
            nc.scalar.activation(
                rw[:], lnt[:], mybir.ActivationFunctionType.Exp, bias=zero_b[:], scale=-0.5
            )

            # wn = w * rw  (rw broadcast along k via the partition-bcast trick)
            wn = wns[j]
            for k in range(KB):
                nc.vector.tensor_mul(wn[:, k, :], wtile[:, k, :], rw[:])

            # cos = xn.T @ wn per 128-row batch block; exp + free-axis sum on ACT
            for nb in range(NB):
                pm = psm.tile([128, CH], f32, tag="pm")
                for k in range(KB):
                    nc.tensor.matmul(
                        pm[:],
                        lhsT=xs[:, k, nb * 128 : (nb + 1) * 128],
                        rhs=wn[:, k, :],
                        start=(k == 0),
                        stop=(k == KB - 1),
                    )
                es = escrp.tile([128, CH], bf16, tag="es")
                nc.scalar.activation(
                    es[:],
                    pm[:],
                    mybir.ActivationFunctionType.Exp,
                    bias=nshift_b[:],
                    scale=S,
                    accum_out=s_parts[:, nb * NCH + j : nb * NCH + j + 1],
                )

        s_fin = singles.tile([128, NB], f32)
        nc.vector.tensor_reduce(
            out=s_fin[:],
            in_=s_parts[:].rearrange("p (nb nch) -> p nb nch", nb=NB),
            axis=mybir.AxisListType.X,
            op=mybir.AluOpType.add,
        )
        nc.sync.dma_start(out=s_out.rearrange("nb p -> p nb"), in_=s_fin[:])

    nc.compile()
    return nc


def _get_nc():
    if "nc" not in _CACHE:
        _CACHE["nc"] = _build_nc()
    return _CACHE["nc"]


def _prep_inputs(x, weights):
    """Host-side shard/layout prep: normalize x, shard+transpose+cast W."""
    x = np.asarray(x, dtype=np.float32)
    w = np.asarray(weights, dtype=np.float32)

    xn = x / np.linalg.norm(x.astype(np.float64), axis=1, keepdims=True)
    xnt = np.ascontiguousarray(xn.T).astype(ml_dtypes.bfloat16)

    wpad = np.zeros((CPAD, D), dtype=np.float32)
    wpad[:C] = w
    wt_maps = []
    for i in range(NCORES):
        shard = wpad[i * CSH : (i + 1) * CSH]  # [12800, 512]
        # -> [p, j, k, c] with [j,k,c] contiguous per partition
        arr = shard.reshape(NCH, CH, KB, 128).transpose(3, 0, 2, 1)
        wt_maps.append(np.ascontiguousarray(arr).astype(ml_dtypes.bfloat16))
    return xnt, wt_maps


def _run_on_device(xnt, wt_maps, trace=False):
    from concourse.bass_utils import run_bass_kernel_spmd

    nc = _get_nc()
    in_maps = [{"xnt": xnt, "wt": wt_maps[i]} for i in range(NCORES)]
    res = run_bass_kernel_spmd(
        nc, in_maps, core_ids=list(range(NCORES)), trace=trace
    )
    _CACHE["last_results"] = res
    return [r["s_out"].reshape(B).astype(np.float64) for r in res.results]


def kernel(x, weights, targets, _trace=False):
    x = np.asarray(x)
    weights = np.asarray(weights)
    targets = np.asarray(targets).astype(np.int64)

    xnt, wt_maps = _prep_inputs(x, weights)
    s_shards = _run_on_device(xnt, wt_maps, trace=_trace)

    # ---- host combine (f64, ~0.5 MFLOP total) ----
    s_total = np.sum(s_shards, axis=0)  # [B]
    # remove zero-pad classes: each contributes exp(0*S - SHIFT) exactly
    npad = CPAD - C
    s_total = s_total - npad * math.exp(-SHIFT)

    xf = x.astype(np.float64)
    xn = xf / np.linalg.norm(xf, axis=1, keepdims=True)
    wtg = weights.astype(np.float64)[targets]  # [B, D] gathered target rows
    wtg = wtg / np.linalg.norm(wtg, axis=1, keepdims=True)
    cos_t = np.einsum("bd,bd->b", xn, wtg)

    sin_t = np.sqrt(np.clip(1.0 - cos_t * cos_t, 0.0, 1.0))
    phi = cos_t * COS_M - sin_t * SIN_M
    psi = np.where(cos_t > TH, phi, cos_t - MM)

    # swap the target term: remove exp(S*cos_t), add exp(S*psi)
    s_adj = s_total - np.exp(S * cos_t - SHIFT) + np.exp(S * psi - SHIFT)
    lse = SHIFT + np.log(s_adj)
    loss = np.mean(lse - S * psi)
    return np.float32(loss)


# revision 16
# speedup vs baseline: 1.1356x; 1.1356x over previous
"""ArcFace loss kernel for Trainium2, class-sharded across 8 NeuronCores.

Strategy (vocab/tensor parallel per the module's own sharding):
  - Shard the class axis of `weights` (100000 classes -> 8 x 12800, zero-padded).
  - Each core: normalize its weight shard on device (squares -> ones-matmul
    partition-reduce -> rsqrt via exp(-0.5*ln)), then bf16 matmul
    cos[b, c] = xn . wn with x replicated, then ScalarE exp(64*cos - 64)
    with accum_out producing per-row partial sums of exp.
  - Host: sum the 8 partial exp-sums (f64), fix up the 512 target-class
    entries with the ArcFace margin (cos(theta+m) correction), and take the
    mean cross-entropy.  A fixed shift of -64 (= -S, since cos <= 1) replaces
    the usual running max, so no cross-core max reduction is needed; all
    exp values stay inside normal fp32 range.
"""

import math

import ml_dtypes
import numpy as np

# Problem constants (hardcoded per contract; kernel.py must be self-contained).
B = 512  # batch
D = 512  # feature dim
C = 100000  # classes
S = 64.0
MARGIN = 0.5
COS_M = math.cos(MARGIN)
SIN_M = math.sin(MARGIN)
TH = math.cos(math.pi - MARGIN)
MM = math.sin(math.pi - MARGIN) * MARGIN

NCORES = 8
CH = 512  # classes per chunk (one PSUM bank of fp32)
NCH = 25  # chunks per core
CSH = CH * NCH  # 12800 padded classes per core
CPAD = CSH * NCORES  # 102400
KB = D // 128  # 4 contraction blocks
NB = B // 128  # 4 batch blocks
SHIFT = 64.0  # fixed logsumexp shift (logits = S*cos <= 64)

_CACHE = {}


def _fix_act_tables():
    """Make both Exp and Ln resolve to the one table set containing both.

    bass picks the first act-function set containing a needed function; by
    default Exp -> 'exp_and_others' and Ln -> 'natural_log' which thrashes the
    ACT table RAMs (~1.3us per reload, dozens of reloads).  Blank those two
    sets in the cached map (same dict object is returned every call) so both
    functions resolve to 'natural_log_exp_and_others'.  Set *indices* are
    untouched, so the act_func_set_id stays consistent with act_info.json.
    """
    import concourse.hw_specs as hw_specs

    tables = hw_specs.get_activation_tables("gen3")
    for name in ("exp_and_others", "natural_log"):
        if name in tables and "natural_log_exp_and_others" in tables:
            tables[name].clear()


def _build_nc(repeat=1):
    import concourse.bass as bass
    import concourse.tile as tile
    from concourse import bacc, mybir

    _fix_act_tables()
    nc = bacc.Bacc(
        "TRN2",
        target_bir_lowering=False,
        debug=False,
        enable_asserts=False,
        num_devices=NCORES,
    )
    bf16 = mybir.dt.bfloat16
    f32 = mybir.dt.float32

    # xnt[d, b] = normalized-x transposed; wt[p, j, k, c] = wn-shard laid out so
    # each 512-class chunk is one contiguous 4KB run per partition.
    xnt = nc.dram_tensor("xnt", [D, B], bf16, kind="ExternalInput").ap()
    wt = nc.dram_tensor("wt", [128, NCH, KB, CH], bf16, kind="ExternalInput").ap()
    s_out = nc.dram_tensor("s_out", [NB, 128], f32, kind="ExternalOutput").ap()

    from contextlib import ExitStack, nullcontext

    with tile.TileContext(nc) as tc, ExitStack() as ctx:
        singles = ctx.enter_context(tc.tile_pool(name="singles", bufs=1))
        wpool = ctx.enter_context(tc.tile_pool(name="wpool", bufs=5))
        qpool = ctx.enter_context(tc.tile_pool(name="qpool", bufs=3))
        rwpool = ctx.enter_context(tc.tile_pool(name="rwpool", bufs=3))
        tpool = ctx.enter_context(tc.tile_pool(name="tpool", bufs=3))
        escrp = ctx.enter_context(tc.tile_pool(name="escr", bufs=4))
        wnpool = ctx.enter_context(tc.tile_pool(name="wnpool", bufs=1))
        psn = ctx.enter_context(tc.tile_pool(name="psn", bufs=2, space="PSUM"))
        psm = ctx.enter_context(tc.tile_pool(name="psm", bufs=2, space="PSUM"))

        ctx.enter_context(tc.For_i(0, repeat, 1) if repeat > 1 else nullcontext())

        # x (stationary operand of every matmul): [p, k, b], b-contiguous.
        xs = singles.tile([128, KB, B], bf16)
        nc.sync.dma_start(out=xs[:], in_=xnt.rearrange("(k p) b -> p k b", p=128))

        ones_t = singles.tile([128, 128], bf16)
        nc.vector.memset(ones_t[:], 1.0)

        # per-partition bias vectors (activation bias must be an AP)
        eps_b = singles.tile([128, 1], f32)
        nc.vector.memset(eps_b[:], 1e-12)
        zero_b = singles.tile([128, 1], f32)
        nc.vector.memset(zero_b[:], 0.0)
        nshift_b = singles.tile([128, 1], f32)
        nc.vector.memset(nshift_b[:], -SHIFT)

        # supers: groups of chunks sharing one multi-bank PSUM tile + one exp
        supers = []
        c0 = 0
        while c0 < NCH:
            n = min(3, NCH - c0)
            supers.append(list(range(c0, c0 + n)))
            c0 += n
        NSUP = len(supers)

        # accum_out landing area: one f32 scalar per (batch row, super).
        s_parts = singles.tile([128, NB * NSUP], f32)

        # Per-chunk resident normalized weights (distinct tiles -> no false WAR).
        wns = [
            wnpool.tile([128, KB, CH], bf16, tag=f"wn{j}", name=f"wn{j}")
            for j in range(NCH)
        ]

        for si, sup in enumerate(supers):
            ns = len(sup)
            # ---- produce normalized weights for this super's chunks ----
            for j in sup:
                wtile = wpool.tile([128, KB, CH], bf16, tag="wt", name=f"wt{j}")
                nc.sync.dma_start(out=wtile[:], in_=wt[:, j, :, :])

                # q = w*w (bf16: norm err ~0.4%/sqrt(512))
                q = qpool.tile([128, KB, CH], bf16, tag="q", name=f"q{j}")
                nc.vector.tensor_mul(q[:], wtile[:], wtile[:])

                # n2[c] broadcast to all 128 partitions via ones.T @ q
                pn = psn.tile([128, CH], f32, tag="pn", name=f"pn{j}")
                for k in range(KB):
                    nc.tensor.matmul(
                        pn[:],
                        lhsT=ones_t[:],
                        rhs=q[:, k, :],
                        start=(k == 0),
                        stop=(k == KB - 1),
                    )

                # rw = (n2+eps)^-0.5 = exp(-0.5*ln(n2+eps)); Ln+Exp share one
                # ACT table set (see _fix_act_tables) -> no table thrash.
                lnt = tpool.tile([128, CH], f32, tag="lnt", name=f"lnt{j}")
                nc.scalar.activation(
                    lnt[:],
                    pn[:],
                    mybir.ActivationFunctionType.Ln,
                    bias=eps_b[:],
                    scale=1.0,
                )
                rw = rwpool.tile([128, CH], bf16, tag="rw", name=f"rw{j}")
                nc.scalar.activation(
                    rw[:],
                    lnt[:],
                    mybir.ActivationFunctionType.Exp,
                    bias=zero_b[:],
                    scale=-0.5,
                )

                # wn = w * rw (one DVE op; rw broadcast over k via step-0 AP)
                rw_b = bass.AP(
                    tensor=rw.tensor,
                    offset=rw.offset,
                    ap=[rw.ap[0], [0, KB], rw.ap[1]],
                )
                nc.vector.tensor_mul(wns[j][:], wtile[:], rw_b)

            # ---- logits + exp for this super across all batch blocks ----
            for nb in range(NB):
                pm = psm.tile([128, 3 * CH], f32, tag="pm", name=f"pm{si}_{nb}")
                for ci, j in enumerate(sup):
                    for k in range(KB):
                        nc.tensor.matmul(
                            pm[:, ci * CH : (ci + 1) * CH],
                            lhsT=xs[:, k, nb * 128 : (nb + 1) * 128],
                            rhs=wns[j][:, k, :],
                            start=(k == 0),
                            stop=(k == KB - 1),
                        )
                es = escrp.tile([128, 3 * CH], bf16, tag="es", name=f"es{si}_{nb}")
                nc.scalar.activation(
                    es[:, : ns * CH],
                    pm[:, : ns * CH],
                    mybir.ActivationFunctionType.Exp,
                    bias=nshift_b[:],
                    scale=S,
                    accum_out=s_parts[:, nb * NSUP + si : nb * NSUP + si + 1],
                )

        s_fin = singles.tile([128, NB], f32)
        nc.vector.tensor_reduce(
            out=s_fin[:],
            in_=s_parts[:].rearrange("p (nb nsup) -> p nb nsup", nb=NB),
            axis=mybir.AxisListType.X,
            op=mybir.AluOpType.add,
        )
        nc.sync.dma_start(out=s_out.rearrange("nb p -> p nb"), in_=s_fin[:])

    nc.compile()
    return nc


def _get_nc():
    if "nc" not in _CACHE:
        _CACHE["nc"] = _build_nc()
    return _CACHE["nc"]


def _prep_inputs(x, weights):
    """Host-side shard/layout prep: normalize x, shard+transpose+cast W."""
    x = np.asarray(x, dtype=np.float32)
    w = np.asarray(weights, dtype=np.float32)

    xn = x / np.linalg.norm(x.astype(np.float64), axis=1, keepdims=True)
    xnt = np.ascontiguousarray(xn.T).astype(ml_dtypes.bfloat16)

    wpad = np.zeros((CPAD, D), dtype=np.float32)
    wpad[:C] = w
    wt_maps = []
    for i in range(NCORES):
        shard = wpad[i * CSH : (i + 1) * CSH]  # [12800, 512]
        # -> [p, j, k, c] with [j,k,c] contiguous per partition
        arr = shard.reshape(NCH, CH, KB, 128).transpose(3, 0, 2, 1)
        wt_maps.append(np.ascontiguousarray(arr).astype(ml_dtypes.bfloat16))
    return xnt, wt_maps


def _run_on_device(xnt, wt_maps, trace=False):
    from concourse.bass_utils import run_bass_kernel_spmd

    nc = _get_nc()
    in_maps = [{"xnt": xnt, "wt": wt_maps[i]} for i in range(NCORES)]
    res = run_bass_kernel_spmd(
        nc, in_maps, core_ids=list(range(NCORES)), trace=trace
    )
    _CACHE["last_results"] = res
    return [r["s_out"].reshape(B).astype(np.float64) for r in res.results]


def kernel(x, weights, targets, _trace=False):
    x = np.asarray(x)
    weights = np.asarray(weights)
    targets = np.asarray(targets).astype(np.int64)

    xnt, wt_maps = _prep_inputs(x, weights)
    s_shards = _run_on_device(xnt, wt_maps, trace=_trace)

    # ---- host combine (f64, ~0.5 MFLOP total) ----
    s_total = np.sum(s_shards, axis=0)  # [B]
    # remove zero-pad classes: each contributes exp(0*S - SHIFT) exactly
    npad = CPAD - C
    s_total = s_total - npad * math.exp(-SHIFT)

    xf = x.astype(np.float64)
    xn = xf / np.linalg.norm(xf, axis=1, keepdims=True)
    wtg = weights.astype(np.float64)[targets]  # [B, D] gathered target rows
    wtg = wtg / np.linalg.norm(wtg, axis=1, keepdims=True)
    cos_t = np.einsum("bd,bd->b", xn, wtg)

    sin_t = np.sqrt(np.clip(1.0 - cos_t * cos_t, 0.0, 1.0))
    phi = cos_t * COS_M - sin_t * SIN_M
    psi = np.where(cos_t > TH, phi, cos_t - MM)

    # swap the target term: remove exp(S*cos_t), add exp(S*psi)
    s_adj = s_total - np.exp(S * cos_t - SHIFT) + np.exp(S * psi - SHIFT)
    lse = SHIFT + np.log(s_adj)
    loss = np.mean(lse - S * psi)
    return np.float32(loss)


# revision 18
# speedup vs baseline: 1.1861x; 1.0445x over previous
"""ArcFace loss kernel for Trainium2, class-sharded across 8 NeuronCores.

Strategy (vocab/tensor parallel per the module's own sharding):
  - Shard the class axis of `weights` (100000 classes -> 8 x 12800, zero-padded).
  - Each core: normalize its weight shard on device (squares -> ones-matmul
    partition-reduce -> rsqrt via exp(-0.5*ln)), then bf16 matmul
    cos[b, c] = xn . wn with x replicated, then ScalarE exp(64*cos - 64)
    with accum_out producing per-row partial sums of exp.
  - Host: sum the 8 partial exp-sums (f64), fix up the 512 target-class
    entries with the ArcFace margin (cos(theta+m) correction), and take the
    mean cross-entropy.  A fixed shift of -64 (= -S, since cos <= 1) replaces
    the usual running max, so no cross-core max reduction is needed; all
    exp values stay inside normal fp32 range.
"""

import math

import ml_dtypes
import numpy as np

# Problem constants (hardcoded per contract; kernel.py must be self-contained).
B = 512  # batch
D = 512  # feature dim
C = 100000  # classes
S = 64.0
MARGIN = 0.5
COS_M = math.cos(MARGIN)
SIN_M = math.sin(MARGIN)
TH = math.cos(math.pi - MARGIN)
MM = math.sin(math.pi - MARGIN) * MARGIN

NCORES = 8
CH = 512  # classes per chunk (one PSUM bank of fp32)
NCH = 25  # chunks per core
CSH = CH * NCH  # 12800 padded classes per core
CPAD = CSH * NCORES  # 102400
KB = D // 128  # 4 contraction blocks
NB = B // 128  # 4 batch blocks
SHIFT = 64.0  # fixed logsumexp shift (logits = S*cos <= 64)

_CACHE = {}


def _fix_act_tables():
    """Make both Exp and Ln resolve to the one table set containing both.

    bass picks the first act-function set containing a needed function; by
    default Exp -> 'exp_and_others' and Ln -> 'natural_log' which thrashes the
    ACT table RAMs (~1.3us per reload, dozens of reloads).  Blank those two
    sets in the cached map (same dict object is returned every call) so both
    functions resolve to 'natural_log_exp_and_others'.  Set *indices* are
    untouched, so the act_func_set_id stays consistent with act_info.json.
    """
    import concourse.hw_specs as hw_specs

    tables = hw_specs.get_activation_tables("gen3")
    for name in ("exp_and_others", "natural_log"):
        if name in tables and "natural_log_exp_and_others" in tables:
            tables[name].clear()


def _build_nc(repeat=1):
    import concourse.bass as bass
    import concourse.tile as tile
    from concourse import bacc, mybir

    _fix_act_tables()
    nc = bacc.Bacc(
        "TRN2",
        target_bir_lowering=False,
        debug=False,
        enable_asserts=False,
        num_devices=NCORES,
    )
    bf16 = mybir.dt.bfloat16
    f32 = mybir.dt.float32

    # xnt[d, b] = normalized-x transposed; wt[p, j, k, c] = wn-shard laid out so
    # each 512-class chunk is one contiguous 4KB run per partition.
    xnt = nc.dram_tensor("xnt", [D, B], bf16, kind="ExternalInput").ap()
    wt = nc.dram_tensor("wt", [128, NCH, KB, CH], bf16, kind="ExternalInput").ap()
    s_out = nc.dram_tensor("s_out", [NB, 128], f32, kind="ExternalOutput").ap()

    from contextlib import ExitStack, nullcontext

    with tile.TileContext(nc) as tc, ExitStack() as ctx:
        singles = ctx.enter_context(tc.tile_pool(name="singles", bufs=1))
        wpool = ctx.enter_context(tc.tile_pool(name="wpool", bufs=5))
        qpool = ctx.enter_context(tc.tile_pool(name="qpool", bufs=3))
        rwpool = ctx.enter_context(tc.tile_pool(name="rwpool", bufs=3))
        tpool = ctx.enter_context(tc.tile_pool(name="tpool", bufs=3))
        escrp = ctx.enter_context(tc.tile_pool(name="escr", bufs=4))
        wnpool = ctx.enter_context(tc.tile_pool(name="wnpool", bufs=1))
        psn = ctx.enter_context(tc.tile_pool(name="psn", bufs=2, space="PSUM"))
        psm = ctx.enter_context(tc.tile_pool(name="psm", bufs=2, space="PSUM"))

        hint = (
            mybir.EngineType.PE,
            mybir.EngineType.Activation,
            mybir.EngineType.DVE,
            mybir.EngineType.Pool,
            mybir.EngineType.SP,
        )
        ctx.enter_context(
            tc.For_i(0, repeat, 1, hint_engines=hint) if repeat > 1 else nullcontext()
        )

        # x (stationary operand of every matmul): [p, k, b], b-contiguous.
        xs = singles.tile([128, KB, B], bf16)
        nc.sync.dma_start(out=xs[:], in_=xnt.rearrange("(k p) b -> p k b", p=128))

        ones_t = singles.tile([128, 128], bf16)
        nc.vector.memset(ones_t[:], 1.0)

        # per-partition bias vectors (activation bias must be an AP)
        eps_b = singles.tile([128, 1], f32)
        nc.vector.memset(eps_b[:], 1e-12)
        zero_b = singles.tile([128, 1], f32)
        nc.vector.memset(zero_b[:], 0.0)
        nshift_b = singles.tile([128, 1], f32)
        nc.vector.memset(nshift_b[:], -SHIFT)

        # supers: groups of chunks sharing one multi-bank PSUM tile + one exp
        supers = []
        c0 = 0
        while c0 < NCH:
            n = min(3, NCH - c0)
            supers.append(list(range(c0, c0 + n)))
            c0 += n
        NSUP = len(supers)

        # accum_out landing area: one f32 scalar per (batch row, super).
        s_parts = singles.tile([128, NB * NSUP], f32)

        # Per-chunk resident normalized weights (distinct tiles -> no false WAR).
        wns = [
            wnpool.tile([128, KB, CH], bf16, tag=f"wn{j}", name=f"wn{j}")
            for j in range(NCH)
        ]

        for si, sup in enumerate(supers):
            ns = len(sup)
            # ---- produce normalized weights for this super's chunks ----
            for j in sup:
                wtile = wpool.tile([128, KB, CH], bf16, tag="wt", name=f"wt{j}")
                nc.sync.dma_start(out=wtile[:], in_=wt[:, j, :, :])

                # q = w*w (bf16: norm err ~0.4%/sqrt(512))
                q = qpool.tile([128, KB, CH], bf16, tag="q", name=f"q{j}")
                nc.vector.tensor_mul(q[:], wtile[:], wtile[:])

                # n2[c] broadcast to all 128 partitions via ones.T @ q
                pn = psn.tile([128, CH], f32, tag="pn", name=f"pn{j}")
                for k in range(KB):
                    nc.tensor.matmul(
                        pn[:],
                        lhsT=ones_t[:],
                        rhs=q[:, k, :],
                        start=(k == 0),
                        stop=(k == KB - 1),
                    )

                # rw = (n2+eps)^-0.5 = exp(-0.5*ln(n2+eps)); Ln+Exp share one
                # ACT table set (see _fix_act_tables) -> no table thrash.
                lnt = tpool.tile([128, CH], f32, tag="lnt", name=f"lnt{j}")
                nc.scalar.activation(
                    lnt[:],
                    pn[:],
                    mybir.ActivationFunctionType.Ln,
                    bias=eps_b[:],
                    scale=1.0,
                )
                rw = rwpool.tile([128, CH], bf16, tag="rw", name=f"rw{j}")
                nc.scalar.activation(
                    rw[:],
                    lnt[:],
                    mybir.ActivationFunctionType.Exp,
                    bias=zero_b[:],
                    scale=-0.5,
                )

                # wn = w * rw (one DVE op; rw broadcast over k via step-0 AP)
                rw_b = bass.AP(
                    tensor=rw.tensor,
                    offset=rw.offset,
                    ap=[rw.ap[0], [0, KB], rw.ap[1]],
                )
                nc.vector.tensor_mul(wns[j][:], wtile[:], rw_b)

            # ---- logits + exp for this super across all batch blocks ----
            for nb in range(NB):
                pm = psm.tile([128, 3 * CH], f32, tag="pm", name=f"pm{si}_{nb}")
                # k outer: the stationary operand repeats across the chunks of
                # the super, maximizing LDWEIGHTS reuse/overlap
                for k in range(KB):
                    for ci, j in enumerate(sup):
                        nc.tensor.matmul(
                            pm[:, ci * CH : (ci + 1) * CH],
                            lhsT=xs[:, k, nb * 128 : (nb + 1) * 128],
                            rhs=wns[j][:, k, :],
                            start=(k == 0),
                            stop=(k == KB - 1),
                        )
                es = escrp.tile([128, 3 * CH], bf16, tag="es", name=f"es{si}_{nb}")
                nc.scalar.activation(
                    es[:, : ns * CH],
                    pm[:, : ns * CH],
                    mybir.ActivationFunctionType.Exp,
                    bias=nshift_b[:],
                    scale=S,
                    accum_out=s_parts[:, nb * NSUP + si : nb * NSUP + si + 1],
                )

        s_fin = singles.tile([128, NB], f32)
        nc.vector.tensor_reduce(
            out=s_fin[:],
            in_=s_parts[:].rearrange("p (nb nsup) -> p nb nsup", nb=NB),
            axis=mybir.AxisListType.X,
            op=mybir.AluOpType.add,
        )
        nc.sync.dma_start(out=s_out.rearrange("nb p -> p nb"), in_=s_fin[:])

    nc.compile()
    return nc


def _get_nc():
    if "nc" not in _CACHE:
        _CACHE["nc"] = _build_nc()
    return _CACHE["nc"]


def _prep_inputs(x, weights):
    """Host-side shard/layout prep: normalize x, shard+transpose+cast W."""
    x = np.asarray(x, dtype=np.float32)
    w = np.asarray(weights, dtype=np.float32)

    xn = x / np.linalg.norm(x.astype(np.float64), axis=1, keepdims=True)
    xnt = np.ascontiguousarray(xn.T).astype(ml_dtypes.bfloat16)

    wpad = np.zeros((CPAD, D), dtype=np.float32)
    wpad[:C] = w
    wt_maps = []
    for i in range(NCORES):
        shard = wpad[i * CSH : (i + 1) * CSH]  # [12800, 512]
        # -> [p, j, k, c] with [j,k,c] contiguous per partition
        arr = shard.reshape(NCH, CH, KB, 128).transpose(3, 0, 2, 1)
        wt_maps.append(np.ascontiguousarray(arr).astype(ml_dtypes.bfloat16))
    return xnt, wt_maps


def _run_on_device(xnt, wt_maps, trace=False):
    from concourse.bass_utils import run_bass_kernel_spmd

    nc = _get_nc()
    in_maps = [{"xnt": xnt, "wt": wt_maps[i]} for i in range(NCORES)]
    res = run_bass_kernel_spmd(
        nc, in_maps, core_ids=list(range(NCORES)), trace=trace
    )
    _CACHE["last_results"] = res
    return [r["s_out"].reshape(B).astype(np.float64) for r in res.results]


def kernel(x, weights, targets, _trace=False):
    x = np.asarray(x)
    weights = np.asarray(weights)
    targets = np.asarray(targets).astype(np.int64)

    xnt, wt_maps = _prep_inputs(x, weights)
    s_shards = _run_on_device(xnt, wt_maps, trace=_trace)

    # ---- host combine (f64, ~0.5 MFLOP total) ----
    s_total = np.sum(s_shards, axis=0)  # [B]
    # remove zero-pad classes: each contributes exp(0*S - SHIFT) exactly
    npad = CPAD - C
    s_total = s_total - npad * math.exp(-SHIFT)

    xf = x.astype(np.float64)
    xn = xf / np.linalg.norm(xf, axis=1, keepdims=True)
    wtg = weights.astype(np.float64)[targets]  # [B, D] gathered target rows
    wtg = wtg / np.linalg.norm(wtg, axis=1, keepdims=True)
    cos_t = np.einsum("bd,bd->b", xn, wtg)

    sin_t = np.sqrt(np.clip(1.0 - cos_t * cos_t, 0.0, 1.0))
    phi = cos_t * COS_M - sin_t * SIN_M
    psi = np.where(cos_t > TH, phi, cos_t - MM)

    # swap the target term: remove exp(S*cos_t), add exp(S*psi)
    s_adj = s_total - np.exp(S * cos_t - SHIFT) + np.exp(S * psi - SHIFT)
    lse = SHIFT + np.log(s_adj)
    loss = np.mean(lse - S * psi)
    return np.float32(loss)


# revision 24
# speedup vs baseline: 1.2101x; 1.0202x over previous
"""ArcFace loss kernel for Trainium2, class-sharded across 8 NeuronCores.

Strategy (vocab/tensor parallel per the module's own sharding):
  - Shard the class axis of `weights` (100000 classes -> 8 x 12800, zero-padded).
  - Each core: normalize its weight shard on device (squares -> ones-matmul
    partition-reduce -> rsqrt via exp(-0.5*ln)), then bf16 matmul
    cos[b, c] = xn . wn with x replicated, then ScalarE exp(64*cos - 64)
    with accum_out producing per-row partial sums of exp.
  - Host: sum the 8 partial exp-sums (f64), fix up the 512 target-class
    entries with the ArcFace margin (cos(theta+m) correction), and take the
    mean cross-entropy.  A fixed shift of -64 (= -S, since cos <= 1) replaces
    the usual running max, so no cross-core max reduction is needed; all
    exp values stay inside normal fp32 range.
"""

import math

import ml_dtypes
import numpy as np

# Problem constants (hardcoded per contract; kernel.py must be self-contained).
B = 512  # batch
D = 512  # feature dim
C = 100000  # classes
S = 64.0
MARGIN = 0.5
COS_M = math.cos(MARGIN)
SIN_M = math.sin(MARGIN)
TH = math.cos(math.pi - MARGIN)
MM = math.sin(math.pi - MARGIN) * MARGIN

NCORES = 8
CH = 512  # classes per chunk (one PSUM bank of fp32)
NCH = 25  # chunks per core
CSH = CH * NCH  # 12800 padded classes per core
CPAD = CSH * NCORES  # 102400
KB = D // 128  # 4 contraction blocks
NB = B // 128  # 4 batch blocks
SHIFT = 64.0  # fixed logsumexp shift (logits = S*cos <= 64)

_CACHE = {}


def _fix_act_tables():
    """Make both Exp and Ln resolve to the one table set containing both.

    bass picks the first act-function set containing a needed function; by
    default Exp -> 'exp_and_others' and Ln -> 'natural_log' which thrashes the
    ACT table RAMs (~1.3us per reload, dozens of reloads).  Blank those two
    sets in the cached map (same dict object is returned every call) so both
    functions resolve to 'natural_log_exp_and_others'.  Set *indices* are
    untouched, so the act_func_set_id stays consistent with act_info.json.
    """
    import concourse.hw_specs as hw_specs

    tables = hw_specs.get_activation_tables("gen3")
    for name in ("exp_and_others", "natural_log"):
        if name in tables and "natural_log_exp_and_others" in tables:
            tables[name].clear()


def _build_nc(repeat=1, mm_order="ci_inner", host_norm=False, sup_n=3, pm_bufs=2,
              super_dma=False, q_fold=False, exp_inplace=False, split_first=False):
    import concourse.bass as bass
    import concourse.tile as tile
    from concourse import bacc, mybir

    _fix_act_tables()
    nc = bacc.Bacc(
        "TRN2",
        target_bir_lowering=False,
        debug=False,
        enable_asserts=False,
        num_devices=NCORES,
    )
    bf16 = mybir.dt.bfloat16
    f32 = mybir.dt.float32

    # xnt[d, b] = normalized-x transposed; wt[p, j, k, c] = wn-shard laid out so
    # each 512-class chunk is one contiguous 4KB run per partition.
    xnt = nc.dram_tensor("xnt", [D, B], bf16, kind="ExternalInput").ap()
    wt = nc.dram_tensor("wt", [128, NCH, KB, CH], bf16, kind="ExternalInput").ap()
    s_out = nc.dram_tensor("s_out", [NB, 128], f32, kind="ExternalOutput").ap()

    from contextlib import ExitStack, nullcontext

    with tile.TileContext(nc) as tc, ExitStack() as ctx:
        singles = ctx.enter_context(tc.tile_pool(name="singles", bufs=1))
        wpool = ctx.enter_context(tc.tile_pool(name="wpool", bufs=5))
        qpool = ctx.enter_context(tc.tile_pool(name="qpool", bufs=3))
        rwpool = ctx.enter_context(tc.tile_pool(name="rwpool", bufs=3))
        tpool = ctx.enter_context(tc.tile_pool(name="tpool", bufs=3))
        escrp = ctx.enter_context(tc.tile_pool(name="escr", bufs=4))
        wnpool = ctx.enter_context(tc.tile_pool(name="wnpool", bufs=1))
        psn = ctx.enter_context(tc.tile_pool(name="psn", bufs=2, space="PSUM"))
        psm = ctx.enter_context(tc.tile_pool(name="psm", bufs=pm_bufs, space="PSUM"))

        hint = (
            mybir.EngineType.PE,
            mybir.EngineType.Activation,
            mybir.EngineType.DVE,
            mybir.EngineType.Pool,
            mybir.EngineType.SP,
        )
        ctx.enter_context(
            tc.For_i(0, repeat, 1, hint_engines=hint) if repeat > 1 else nullcontext()
        )

        # x (stationary operand of every matmul): [p, k, b], b-contiguous.
        xs = singles.tile([128, KB, B], bf16)
        nc.sync.dma_start(out=xs[:], in_=xnt.rearrange("(k p) b -> p k b", p=128))

        ones_t = singles.tile([128, 128], bf16)
        nc.vector.memset(ones_t[:], 1.0)

        # per-partition bias vectors (activation bias must be an AP)
        eps_b = singles.tile([128, 1], f32)
        nc.vector.memset(eps_b[:], 1e-12)
        zero_b = singles.tile([128, 1], f32)
        nc.vector.memset(zero_b[:], 0.0)
        nshift_b = singles.tile([128, 1], f32)
        nc.vector.memset(nshift_b[:], -SHIFT)

        # supers: groups of chunks sharing one multi-bank PSUM tile + one exp
        supers = []
        c0 = 0
        while c0 < NCH:
            n = min(sup_n, NCH - c0)
            supers.append(list(range(c0, c0 + n)))
            c0 += n
        NSUP = len(supers)

        # accum_out landing area: one f32 scalar per (batch row, super).
        s_parts = singles.tile([128, NB * NSUP], f32)

        # Per-chunk resident normalized weights (distinct tiles -> no false WAR).
        wns = [
            wnpool.tile([128, KB, CH], bf16, tag=f"wn{j}", name=f"wn{j}")
            for j in range(NCH)
        ]

        for si, sup in enumerate(supers):
            ns = len(sup)
            # ---- produce normalized weights for this super's chunks ----
            wsup = None
            if super_dma and not host_norm:
                wsup = wpool.tile([128, len(sup), KB, CH], bf16, tag="wt",
                                  name=f"wsup{si}", padded_shape=[128, sup_n, KB, CH])
                nc.sync.dma_start(out=wsup[:], in_=wt[:, sup[0] : sup[0] + ns, :, :])
            for ji, j in enumerate(sup):
                if host_norm:
                    # diagnostic variant: wt arrives pre-normalized from host
                    nc.sync.dma_start(out=wns[j][:], in_=wt[:, j, :, :])
                    continue
                if wsup is not None:
                    wtile = wsup[:, ji, :, :]
                else:
                    wtile = wpool.tile([128, KB, CH], bf16, tag="wt", name=f"wt{j}")
                    if split_first and j == 0:
                        for k in range(KB):
                            nc.sync.dma_start(
                                out=wtile[:, k, :], in_=wt[:, j, k, :]
                            )
                    else:
                        nc.sync.dma_start(out=wtile[:], in_=wt[:, j, :, :])

                # q = w*w (bf16: norm err ~0.4%/sqrt(512))
                q = qpool.tile([128, KB, CH], bf16, tag="q", name=f"q{j}")
                nc.vector.tensor_mul(q[:], wtile[:], wtile[:])

                if q_fold:
                    # fold the 4 k-planes on DVE -> single norm matmul (K=128)
                    qa = qpool.tile([128, 2, CH], bf16, tag="qa", name=f"qa{j}")
                    nc.vector.tensor_add(qa[:, 0, :], q[:, 0, :], q[:, 1, :])
                    nc.vector.tensor_add(qa[:, 1, :], q[:, 2, :], q[:, 3, :])
                    qf = qpool.tile([128, CH], bf16, tag="qf", name=f"qf{j}")
                    nc.vector.tensor_add(qf[:], qa[:, 0, :], qa[:, 1, :])
                    pn = psn.tile([128, CH], f32, tag="pn", name=f"pn{j}")
                    nc.tensor.matmul(pn[:], lhsT=ones_t[:], rhs=qf[:], start=True, stop=True)
                else:
                    # n2[c] broadcast to all 128 partitions via ones.T @ q
                    pn = psn.tile([128, CH], f32, tag="pn", name=f"pn{j}")
                    for k in range(KB):
                        nc.tensor.matmul(
                            pn[:],
                            lhsT=ones_t[:],
                            rhs=q[:, k, :],
                            start=(k == 0),
                            stop=(k == KB - 1),
                        )

                # rw = (n2+eps)^-0.5 = exp(-0.5*ln(n2+eps)); Ln+Exp share one
                # ACT table set (see _fix_act_tables) -> no table thrash.
                lnt = tpool.tile([128, CH], f32, tag="lnt", name=f"lnt{j}")
                nc.scalar.activation(
                    lnt[:],
                    pn[:],
                    mybir.ActivationFunctionType.Ln,
                    bias=eps_b[:],
                    scale=1.0,
                )
                rw = rwpool.tile([128, CH], bf16, tag="rw", name=f"rw{j}")
                nc.scalar.activation(
                    rw[:],
                    lnt[:],
                    mybir.ActivationFunctionType.Exp,
                    bias=zero_b[:],
                    scale=-0.5,
                )

                # wn = w * rw (one DVE op; rw broadcast over k via step-0 AP)
                rw_b = bass.AP(
                    tensor=rw.tensor,
                    offset=rw.offset,
                    ap=[rw.ap[0], [0, KB], rw.ap[1]],
                )
                nc.vector.tensor_mul(wns[j][:], wtile[:], rw_b)

            # ---- logits + exp for this super across all batch blocks ----
            for nb in range(NB):
                pm = psm.tile([128, sup_n * CH], f32, tag="pm", name=f"pm{si}_{nb}")
                # k outer: the stationary operand repeats across the chunks of
                # the super, maximizing LDWEIGHTS reuse/overlap
                if mm_order == "k_outer":
                    for k in range(KB):
                        for ci, j in enumerate(sup):
                            nc.tensor.matmul(
                                pm[:, ci * CH : (ci + 1) * CH],
                                lhsT=xs[:, k, nb * 128 : (nb + 1) * 128],
                                rhs=wns[j][:, k, :],
                                start=(k == 0),
                                stop=(k == KB - 1),
                            )
                else:
                    for ci, j in enumerate(sup):
                        for k in range(KB):
                            nc.tensor.matmul(
                                pm[:, ci * CH : (ci + 1) * CH],
                                lhsT=xs[:, k, nb * 128 : (nb + 1) * 128],
                                rhs=wns[j][:, k, :],
                                start=(k == 0),
                                stop=(k == KB - 1),
                            )
                if exp_inplace:
                    nc.scalar.activation(
                        pm[:, : ns * CH],
                        pm[:, : ns * CH],
                        mybir.ActivationFunctionType.Exp,
                        bias=nshift_b[:],
                        scale=S,
                        accum_out=s_parts[:, nb * NSUP + si : nb * NSUP + si + 1],
                    )
                else:
                    es = escrp.tile([128, sup_n * CH], bf16, tag="es", name=f"es{si}_{nb}")
                    nc.scalar.activation(
                        es[:, : ns * CH],
                        pm[:, : ns * CH],
                        mybir.ActivationFunctionType.Exp,
                        bias=nshift_b[:],
                        scale=S,
                        accum_out=s_parts[:, nb * NSUP + si : nb * NSUP + si + 1],
                    )

        s_fin = singles.tile([128, NB], f32)
        nc.vector.tensor_reduce(
            out=s_fin[:],
            in_=s_parts[:].rearrange("p (nb nsup) -> p nb nsup", nb=NB),
            axis=mybir.AxisListType.X,
            op=mybir.AluOpType.add,
        )
        nc.sync.dma_start(out=s_out.rearrange("nb p -> p nb"), in_=s_fin[:])

    nc.compile()
    return nc


def _get_nc():
    if "nc" not in _CACHE:
        _CACHE["nc"] = _build_nc()
    return _CACHE["nc"]


def _prep_inputs(x, weights):
    """Host-side shard/layout prep: normalize x, shard+transpose+cast W."""
    x = np.asarray(x, dtype=np.float32)
    w = np.asarray(weights, dtype=np.float32)

    xn = x / np.linalg.norm(x.astype(np.float64), axis=1, keepdims=True)
    xnt = np.ascontiguousarray(xn.T).astype(ml_dtypes.bfloat16)

    wpad = np.zeros((CPAD, D), dtype=np.float32)
    wpad[:C] = w
    wt_maps = []
    for i in range(NCORES):
        shard = wpad[i * CSH : (i + 1) * CSH]  # [12800, 512]
        # -> [p, j, k, c] with [j,k,c] contiguous per partition
        arr = shard.reshape(NCH, CH, KB, 128).transpose(3, 0, 2, 1)
        wt_maps.append(np.ascontiguousarray(arr).astype(ml_dtypes.bfloat16))
    return xnt, wt_maps


def _run_on_device(xnt, wt_maps, trace=False):
    from concourse.bass_utils import run_bass_kernel_spmd

    nc = _get_nc()
    in_maps = [{"xnt": xnt, "wt": wt_maps[i]} for i in range(NCORES)]
    res = run_bass_kernel_spmd(
        nc, in_maps, core_ids=list(range(NCORES)), trace=trace
    )
    _CACHE["last_results"] = res
    return [r["s_out"].reshape(B).astype(np.float64) for r in res.results]


def kernel(x, weights, targets, _trace=False):
    x = np.asarray(x)
    weights = np.asarray(weights)
    targets = np.asarray(targets).astype(np.int64)

    xnt, wt_maps = _prep_inputs(x, weights)
    s_shards = _run_on_device(xnt, wt_maps, trace=_trace)

    # ---- host combine (f64, ~0.5 MFLOP total) ----
    s_total = np.sum(s_shards, axis=0)  # [B]
    # remove zero-pad classes: each contributes exp(0*S - SHIFT) exactly
    npad = CPAD - C
    s_total = s_total - npad * math.exp(-SHIFT)

    xf = x.astype(np.float64)
    xn = xf / np.linalg.norm(xf, axis=1, keepdims=True)
    wtg = weights.astype(np.float64)[targets]  # [B, D] gathered target rows
    wtg = wtg / np.linalg.norm(wtg, axis=1, keepdims=True)
    cos_t = np.einsum("bd,bd->b", xn, wtg)

    sin_t = np.sqrt(np.clip(1.0 - cos_t * cos_t, 0.0, 1.0))
    phi = cos_t * COS_M - sin_t * SIN_M
    psi = np.where(cos_t > TH, phi, cos_t - MM)

    # swap the target term: remove exp(S*cos_t), add exp(S*psi)
    s_adj = s_total - np.exp(S * cos_t - SHIFT) + np.exp(S * psi - SHIFT)
    lse = SHIFT + np.log(s_adj)
    loss = np.mean(lse - S * psi)
    return np.float32(loss)
